# revision 60
# baseline (speedup 1.0000x reference)
"""MLA prefill kernel v7 for 8 Trainium2 NeuronCores.

Problem: nn_MLA_25967372272133.
  B=2, S=2048, DIM=2048, H=16 heads, q_lora=768, kv_lora=512,
  nope=128, rope=64, v_dim=128, logit softcap 30, causal mask, XSA epilogue.

v7 over the v6 baseline:
  * All matmul operands fp16 (p2/v bf16) -> fast weight load on the PE
    (fp32r weights disabled FWL: LDWEIGHTS was 450us of the 1.0ms).
  * fp16 AllGather (6.8MB wire instead of 13.6MB) fully hidden under the
    replicated A-kv phase.
  * K/V/vT expansion fused into the A-kv loop per 512-token tile: the kv
    latent never round-trips DRAM and lives only in a transient tile.
  * Transposeless XSA epilogue: vT ([d,token] layout) from an extra
    expansion matmul; row sums / self-dots via ones-matmuls; per-column
    scalars broadcast with gpsimd. Replaces ~550 tiny DVE ops + 96 PE
    transposes per batch.
  * Wide [32,2048] rope ops (all-fp16 -> DVE 2x mode).
  * fp16 output partials (host sums in fp32).
"""

import numpy as np
from contextlib import ExitStack

import concourse.bass as bass
import concourse.tile as tile
from concourse import bacc, mybir
from concourse.bass_utils import run_bass_kernel_spmd
from concourse.masks import make_identity

B, S, DIM = 2, 2048, 2048
H, NCORES, HPC = 16, 8, 2
QL, KVL = 768, 512
NOPE, ROPE, VD = 128, 64, 128
QKD = NOPE + ROPE
BS = B * S
SOFTCAP = 30.0
EPS = 1e-6
SCALE = QKD ** -0.5
TT = 512
STRIP = BS // NCORES
AQR = QL + ROPE  # 832 rows gathered per strip

F32 = mybir.dt.float32
F32R = mybir.dt.float32r
F16 = mybir.dt.float16
BF16 = mybir.dt.bfloat16
AX = mybir.AxisListType
OP = mybir.AluOpType
AF = mybir.ActivationFunctionType
RG = [list(range(NCORES))]


def _flat(t):
    return t.rearrange("p a b -> p (a b)")


def _rope6(nc, pool, oute, outo, xe, xo, c, s, n, dt=F16):
    """oute = xe*c - xo*s ; outo = xe*s + xo*c (tiles [32, n])."""
    t1 = pool.tile([32, n], dt, tag="ropet")
    t2 = pool.tile([32, n], dt, tag="ropet")
    nc.vector.tensor_tensor(out=t1, in0=xe, in1=c, op=OP.mult)
    nc.vector.tensor_tensor(out=t2, in0=xo, in1=s, op=OP.mult)
    nc.vector.tensor_tensor(out=oute, in0=t1, in1=t2, op=OP.subtract)
    t3 = pool.tile([32, n], dt, tag="ropet")
    t4 = pool.tile([32, n], dt, tag="ropet")
    nc.vector.tensor_tensor(out=t3, in0=xe, in1=s, op=OP.mult)
    nc.vector.tensor_tensor(out=t4, in0=xo, in1=c, op=OP.mult)
    nc.vector.tensor_tensor(out=outo, in0=t3, in1=t4, op=OP.add)


def _emit(nc, tc, io):
    (xT, xsT, waqT, wakvT, wqbT, wkbT, wvbT, woT,
     cosT, sinT, cossT, sinsT, m4h, out_d) = io

    xT_r = xT.rearrange("(c p) n -> p c n", p=128)

    with (
        tc.tile_pool(name="glob", bufs=1) as glob,
        tc.tile_pool(name="actg", bufs=1) as actg,
        tc.tile_pool(name="scratch", bufs=1, space="DRAM") as dscr,
    ):
        ones_f = glob.tile([128, 1], F32)
        nc.gpsimd.memset(ones_f, 1.0)
        ones_r = glob.tile([128, 1], F32R)
        nc.vector.tensor_copy(ones_r, ones_f)
        ones16 = glob.tile([128, 1], F16)
        nc.vector.tensor_copy(ones16, ones_f)
        eps_sb = glob.tile([1, 1], F32)
        nc.vector.memset(eps_sb, EPS)
        ln16_sb = glob.tile([128, 1], F32)
        nc.vector.memset(ln16_sb, -2.7725887)
        ident_f = glob.tile([128, 128], F32)
        make_identity(nc, ident_f)
        ident16 = glob.tile([128, 128], F16)
        nc.vector.tensor_copy(ident16, ident_f)
        wqb_sb = glob.tile([128, 6, 384], F16)
        wkb_sb = glob.tile([128, 4, 256], F16)
        wvb_sb = glob.tile([128, 4, 256], F16)
        wo_sb = glob.tile([128, 2, DIM], F16)
        m4_sb = glob.tile([128, 8, 512], F16)  # (d-block, head) major on axis 1

        bounce_q = dscr.tile([AQR, STRIP], F16)
        vspill = dscr.tile([128, 16, 2 * VD], F16)   # batch-1 v (token-part) spill
        vinv_d = dscr.tile([B, 2, S], F32)           # 1/||v||^2 per (b, head, token)
        xaG = nc.dram_tensor(
            "xaG", [NCORES * AQR, STRIP], F16, kind="Internal", addr_space="Shared",
        ).ap()

        # phase-B activations, both batches
        kn = [actg.tile([128, 2, S], F16, name=f"kn{b}") for b in range(B)]
        vT = [actg.tile([128, 2, S], F16, name=f"vT{b}") for b in range(B)]
        v_sb = actg.tile([128, 16, 2 * VD], F16, name="v_sb")  # current batch
        qn = [actg.tile([128, 2, S], F16, name=f"qn{b}") for b in range(B)]
        qpe = [actg.tile([64, 2, S], F16, name=f"qpe{b}") for b in range(B)]
        kpe_b = [actg.tile([64, S], F16, name=f"kpe{b}") for b in range(B)]
        y2T = [actg.tile([128, 2, S], F16, name=f"y2T{b}") for b in range(B)]

        _pre = ExitStack()
        kpre = _pre.enter_context(tc.tile_pool(name="pakpre", bufs=1))
        wakv_sb = kpre.tile([128, 16, KVL], F16)
        wakvT_r = wakvT.rearrange("(c p) m -> p c m", p=128)
        for c4 in range(4):
            nc.sync.dma_start(
                out=wakv_sb[:, 4 * c4:4 * (c4 + 1), :],
                in_=wakvT_r[:, 4 * c4:4 * (c4 + 1), :])

        _phase_a_q(nc, tc, xsT, waqT, cossT, sinsT, ones16, eps_sb, bounce_q)
        nc.gpsimd.collective_compute(
            "AllGather", OP.bypass, replica_groups=RG,
            ins=[bounce_q.opt()], outs=[xaG.opt()],
        )
        # deferred global weight loads (phase B consumers)
        nc.sync.dma_start(out=wqb_sb, in_=wqbT.rearrange("(c p) m -> p c m", p=128))
        nc.sync.dma_start(out=wkb_sb, in_=wkbT.rearrange("(c p) m -> p c m", p=128))
        nc.sync.dma_start(out=wvb_sb, in_=wvbT.rearrange("(c p) m -> p c m", p=128))
        nc.sync.dma_start(out=wo_sb, in_=woT.rearrange("(c p) m -> p c m", p=128))
        nc.sync.dma_start(out=_flat(m4_sb), in_=m4h)

        # ---- A-kv (replicated) fused with K/V/vT expansion ----
        with (
            tc.tile_pool(name="pakx", bufs=2) as kxp,
            tc.tile_pool(name="pakr", bufs=2) as krp,
            tc.tile_pool(name="pakv", bufs=2) as kvp,
            tc.tile_pool(name="pakt", bufs=3) as ktp,
            tc.tile_pool(name="pakp", bufs=4, space="PSUM") as pp,
            tc.tile_pool(name="pakq", bufs=1, space="PSUM") as pq,
            tc.tile_pool(name="pake", bufs=2, space="PSUM") as pe_,
        ):
            for t in range(8):
                b, tt = t // 4, t % 4
                tsl = slice(tt * TT, (tt + 1) * TT)
                ps = []
                ssk = pq.tile([1, TT], F32, tag="ssk", name=f"ssk{t}")
                kv_raw = krp.tile([128, 4, TT], F16, tag="kvr", name=f"kvr{t}")
                for half in range(2):
                    xsh = kxp.tile([128, 8, TT], F16, tag="xs", name=f"xs{t}_{half}")
                    nc.sync.dma_start(out=xsh, in_=xT_r[:, 8 * half:8 * (half + 1), t * TT:(t + 1) * TT])
                    for m in range(4):
                        if half == 0:
                            p = pp.tile([128, TT], F32, tag="kp", name=f"kp{t}_{m}")
                            ps.append(p)
                        for kk in range(8):
                            k = 8 * half + kk
                            nc.tensor.matmul(
                                ps[m], wakv_sb[:, k, m * 128:(m + 1) * 128], xsh[:, kk, :],
                                start=(k == 0), stop=(k == 15), skip_group_check=True,
                            )
                for m in range(4):
                    # copy raw latent out first: frees the psum bank without
                    # waiting for the rms chain
                    nc.vector.tensor_copy(kv_raw[:, m, :], ps[m])
                    sq = ktp.tile([128, TT], F16, tag="sq")
                    nc.vector.tensor_tensor(out=sq, in0=kv_raw[:, m, :], in1=kv_raw[:, m, :], op=OP.mult)
                    nc.tensor.matmul(ssk, ones16, sq, start=(m == 0), stop=(m == 3))

                rkv_s = ktp.tile([1, TT], F32, tag="rks")
                nc.scalar.activation(out=rkv_s, in_=ssk, func=AF.Sqrt, scale=1.0 / KVL, bias=eps_sb)
                rkv = ktp.tile([1, TT], F32, tag="rk")
                nc.vector.reciprocal_approx_fast(out=rkv, in_=rkv_s)
                rkv_b = ktp.tile([128, TT], F32, tag="rkb")
                nc.gpsimd.partition_broadcast(rkv_b, rkv)

                kv_t = kvp.tile([128, 4, TT], F16, tag="kvt", name=f"kvt{t}")
                for m in range(4):
                    nc.vector.tensor_tensor(
                        out=kv_t[:, m, :], in0=kv_raw[:, m, :], in1=rkv_b, op=OP.mult)

                # K / vT / v expansion from the transient latent tile
                for h in range(2):
                    kp2 = pe_.tile([128, TT], F32, tag="ep", name=f"ke{t}_{h}")
                    for k in range(4):
                        nc.tensor.matmul(
                            kp2, wkb_sb[:, k, h * 128:(h + 1) * 128], kv_t[:, k, :],
                            start=(k == 0), stop=(k == 3), skip_group_check=True,
                        )
                    nc.vector.tensor_copy(kn[b][:, h, tsl], kp2)
                    vp2 = pe_.tile([128, TT], F32, tag="ep", name=f"ve{t}_{h}")
                    for k in range(4):
                        nc.tensor.matmul(
                            vp2, wvb_sb[:, k, h * 128:(h + 1) * 128], kv_t[:, k, :],
                            start=(k == 0), stop=(k == 3), skip_group_check=True,
                        )
                    nc.scalar.copy(vT[b][:, h, tsl], vp2)
                vstg = None
                if b == 1:
                    vstg = ktp.tile([128, 4, 2 * VD], F16, tag="vstg", name=f"vstg{t}", bufs=2)
                for i in range(4):
                    tc_ = tt * 4 + i
                    vp3 = pe_.tile([128, 2 * VD], F32, tag="epv", name=f"vv{t}_{i}", bufs=1)
                    for k in range(4):
                        nc.tensor.matmul(
                            vp3, kv_t[:, k, i * 128:(i + 1) * 128], wvb_sb[:, k, :],
                            start=(k == 0), stop=(k == 3), skip_group_check=True,
                        )
                    if b == 0:
                        nc.scalar.copy(v_sb[:, tc_, :], vp3)
                    else:
                        nc.scalar.copy(vstg[:, i, :], vp3)
                if b == 1:
                    nc.sync.dma_start(out=vspill[:, 4 * tt:4 * (tt + 1), :], in_=vstg)

        _pre.close()
        _phase_b(nc, tc, ones16, ones_r, ln16_sb, ident16, wqb_sb, wo_sb, m4_sb,
                 cosT, sinT, xaG, kn, vT, v_sb, vspill, vinv_d, qn, qpe, kpe_b,
                 y2T, out_d)


def _phase_a_q(nc, tc, xsT, waqT, cossT, sinsT, ones16, eps_sb, bounce_q):
    """This core's 512-token strip of prenormalized q-latent + roped k_pe."""
    with (
        tc.tile_pool(name="paqw", bufs=1) as wp,
        tc.tile_pool(name="paqs", bufs=1) as sp,
        tc.tile_pool(name="paqt", bufs=2) as tp,
        tc.tile_pool(name="paqp", bufs=1, space="PSUM") as pp,
        tc.tile_pool(name="paqq", bufs=1, space="PSUM") as pq,
    ):
        xs = wp.tile([128, 16, STRIP], F16)
        xsT_r = xsT.rearrange("(c p) n -> p c n", p=128)
        for c4 in range(4):
            nc.sync.dma_start(out=xs[:, 4 * c4:4 * (c4 + 1), :], in_=xsT_r[:, 4 * c4:4 * (c4 + 1), :])
        waqT_r = waqT.rearrange("(c p) m -> p c m", p=128)
        css = wp.tile([32, STRIP], F16)
        nc.sync.dma_start(out=css, in_=cossT)
        sns = wp.tile([32, STRIP], F16)
        nc.sync.dma_start(out=sns, in_=sinsT)

        ps = []
        ssq = pq.tile([1, STRIP], F32, tag="ssq")
        for m in range(7):
            rows = 128 if m < 6 else 64
            wchunk = wp.tile([128, 16, 128], F16, tag="waqc", name=f"waqc{m}", bufs=2)
            nc.sync.dma_start(out=wchunk[:, :, :rows], in_=waqT_r[:, :, m * 128:m * 128 + rows])
            p = pp.tile([128, STRIP], F32, tag=f"qp{m}", name=f"aqp{m}")
            for k in range(16):
                nc.tensor.matmul(
                    p[:rows], wchunk[:, k, :rows], xs[:, k, :],
                    start=(k == 0), stop=(k == 15),
                )
            ps.append(p)
            if m < 6:
                sq = tp.tile([128, STRIP], F16, tag="sq")
                nc.scalar.activation(out=sq, in_=p, func=AF.Square)
                nc.tensor.matmul(ssq, ones16, sq, start=(m == 0), stop=(m == 5))

        rq = sp.tile([1, STRIP], F32)
        nc.scalar.activation(out=rq, in_=ssq, func=AF.Sqrt, scale=1.0 / QL, bias=eps_sb)
        nc.vector.reciprocal(rq, rq)
        rq_b = sp.tile([128, STRIP], F32)
        nc.gpsimd.partition_broadcast(rq_b, rq)

        xa_sb = sp.tile([128, 6, STRIP], F16)
        for m in range(6):
            nc.vector.tensor_tensor(out=xa_sb[:, m, :], in0=ps[m], in1=rq_b, op=OP.mult)
        kpe_sb = sp.tile([ROPE, STRIP], F16)
        _rope6(nc, tp, kpe_sb[0:32, :], kpe_sb[32:64, :],
               ps[6][0:32, :], ps[6][32:64, :], css, sns, STRIP)

        nc.sync.dma_start(
            out=bounce_q[0:QL].rearrange("(c p) n -> p c n", p=128),
            in_=xa_sb)
        nc.sync.dma_start(out=bounce_q[QL:AQR], in_=kpe_sb)


def _phase_b(nc, tc, ones16, ones_r, ln16_sb, ident16, wqb_sb, wo_sb, m4_sb,
             cosT, sinT, xaG, kn, vT, v_sb, vspill, vinv_d, qn, qpe, kpe_b,
             y2T, out_d):
    with (
        tc.tile_pool(name="bfl", bufs=1) as bfl,
        tc.tile_pool(name="bqt", bufs=2) as bqt,
    ):
        qcs = bfl.tile([32, S], F16)
        nc.sync.dma_start(out=qcs, in_=cosT)
        qsn = bfl.tile([32, S], F16)
        nc.sync.dma_start(out=qsn, in_=sinT)

        def kpe_unit(b, psp):
            for tt in range(4):
                s = 4 * b + tt
                nc.sync.dma_start(
                    out=kpe_b[b][:, tt * 512:(tt + 1) * 512],
                    in_=xaG[AQR * s + QL: AQR * s + AQR, :])

        def qproj_unit(b, tt, mpair, psp, xan_t):
            tsl = slice(tt * 512, (tt + 1) * 512)
            for j, m in enumerate(mpair):
                psq = psp[:, j, :]
                for k in range(6):
                    nc.tensor.matmul(
                        psq, wqb_sb[:, k, m * 128:(m + 1) * 128], xan_t[:, k, :],
                        start=(k == 0), stop=(k == 5), skip_group_check=True)
                if m < 2:
                    nc.vector.tensor_copy(qn[b][:, m, tsl], psq)
                else:
                    for h in range(2):
                        _rope6(nc, bqt, qpe[b][0:32, h, tsl], qpe[b][32:64, h, tsl],
                               psq[h * 64:h * 64 + 32, :], psq[h * 64 + 32:h * 64 + 64, :],
                               qcs[:, tsl], qsn[:, tsl], 512)

        def make_qproj_fillers(b):
            units = []
            state = {}
            for tt in range(4):
                def u1(psp, b=b, tt=tt):
                    xan_t = bfl.tile([128, 6, 512], F16, tag="xan", bufs=2,
                                     name=f"xan{b}_{tt}")
                    s = 4 * b + tt
                    nc.sync.dma_start(
                        out=xan_t,
                        in_=xaG[AQR * s: AQR * s + QL, :].rearrange("(c p) n -> p c n", p=128))
                    state[tt] = xan_t
                    qproj_unit(b, tt, [0, 1], psp, xan_t)

                def u2(psp, b=b, tt=tt):
                    qproj_unit(b, tt, [2], psp, state[tt])
                units += [u1, u2]
            return units

        def make_vss_fillers(b):
            units = []
            for h in range(2):
                for half in range(2):
                    def u(psp, b=b, h=h, half=half):
                        for j in range(2):
                            i4 = 2 * half + j
                            tsl = slice(i4 * 512, (i4 + 1) * 512)
                            sqv = bqt.tile([128, 512], F16, tag="sqv")
                            nc.vector.tensor_tensor(
                                out=sqv, in0=vT[b][:, h, tsl], in1=vT[b][:, h, tsl], op=OP.mult)
                            nc.tensor.matmul(psp[0:1, j, :], ones16, sqv,
                                             start=True, stop=True, skip_group_check=True)
                        vtmp = bqt.tile([1, 1024], F32, tag="vtmp")
                        nc.vector.reciprocal_approx_fast(out=vtmp, in_=_flat(psp[0:1, :, :]))
                        nc.sync.dma_start(
                            out=vinv_d[b, h, half * 1024:(half + 1) * 1024], in_=vtmp)
                    units.append(u)
            return units

        def make_wo_fillers(b):
            units = []
            for tc_ in range(16):
                for ocp in range(2):
                    def u(psp, b=b, tc_=tc_, ocp=ocp):
                        for j in range(2):
                            oc = 2 * ocp + j
                            for h in range(2):
                                nc.tensor.matmul(
                                    psp[:, j, :], y2T[b][:, h, tc_ * 128:(tc_ + 1) * 128],
                                    wo_sb[:, h, oc * 512:(oc + 1) * 512],
                                    start=(h == 0), stop=(h == 1), skip_group_check=True)
                        for j in range(2):
                            oc = 2 * ocp + j
                            o_sb = bqt.tile([128, 512], F16, tag="o_sb")
                            if j == 0:
                                nc.vector.tensor_copy(o_sb, psp[:, j, :])
                            else:
                                nc.scalar.copy(o_sb, psp[:, j, :])
                            nc.sync.dma_start(
                                out=out_d[b * S + tc_ * 128: b * S + (tc_ + 1) * 128,
                                          oc * 512:(oc + 1) * 512],
                                in_=o_sb)
                    units.append(u)
            return units

        def make_vload_fillers():
            units = []
            for q4 in range(4):
                def u(psp, q4=q4):
                    nc.sync.dma_start(
                        out=v_sb[:, 4 * q4:4 * (q4 + 1), :],
                        in_=vspill[:, 4 * q4:4 * (q4 + 1), :])
                units.append(u)
            return units

        def attn(b, fillers):
            with (
                tc.tile_pool(name=f"att{b}", bufs=3) as ap,
                tc.tile_pool(name=f"atsm{b}", bufs=1) as smp,
                tc.tile_pool(name=f"atts{b}", bufs=1) as asm,
                tc.tile_pool(name=f"aps{b}", bufs=2, space="PSUM") as aps,
                tc.tile_pool(name=f"apy{b}", bufs=1, space="PSUM") as apy,
                tc.tile_pool(name=f"afil{b}", bufs=1, space="PSUM") as afp,
            ):
                def pop_fill():
                    if fillers:
                        f = fillers.pop(0)
                        psp = afp.tile([128, 2, 512], F32, tag="fill")
                        f(psp)

                for qt in range(4):
                    qsl = slice(qt * 512, (qt + 1) * 512)
                    nkc = 4 * qt + 4
                    py = {}
                    for h in range(2):
                        py[h] = apy.tile([128, 512], F32, tag=f"py{h}", name=f"py_{b}_{qt}_{h}")
                    sums2 = smp.tile([128, 2, 512], F32R, tag="sums2", name=f"sums_{b}_{qt}", bufs=2)
                    vq = smp.tile([1, 2, 512], F32, tag="vq", name=f"vq_{b}_{qt}", bufs=2)
                    nc.sync.dma_start(out=_flat(vq), in_=vinv_d[b, :, qsl])
                    p2s = [None] * nkc

                    def scores(kc, qt=qt, qsl=qsl, sums2=sums2, p2s=p2s):
                        ksl = slice(kc * 128, (kc + 1) * 128)
                        diag = kc >= 4 * qt
                        d = kc - 4 * qt
                        # causal: queries below this block's first key are fully
                        # masked; restrict the q-range instead of masking them
                        qo = d * 128 if diag else 0
                        qsl_r = slice(qt * 512 + qo, (qt + 1) * 512)
                        ps_s = aps.tile([128, 2, 512], F32, tag="ps_s", name=f"ps_{b}_{qt}_{kc}")
                        for h in range(2):
                            nc.tensor.matmul(
                                ps_s[:, h, qo:512], kn[b][:, h, ksl], qn[b][:, h, qsl_r],
                                start=True, stop=False, skip_group_check=True,
                            )
                            nc.tensor.matmul(
                                ps_s[:, h, qo:512], kpe_b[b][:, ksl], qpe[b][:, h, qsl_r],
                                start=False, stop=not diag, skip_group_check=True,
                            )
                            if diag:
                                nc.tensor.matmul(
                                    ps_s[:, h, qo:512], ident16, m4_sb[:, 2 * d + h, qo:512],
                                    start=False, stop=True, skip_group_check=True,
                                )
                        t_sb = ap.tile([128, 2, 512], F32, tag="t_sb", name=f"ts_{b}_{qt}_{kc}", bufs=3)
                        p2 = ap.tile([128, 2, 512], F16, tag="p_sb", name=f"p2_{b}_{qt}_{kc}", bufs=3)
                        # exp(SOFTCAP*t - ln16): 1/16 scale cancels in softmax
                        # normalization; buys fp16 headroom on both range ends.
                        if qo:
                            for h in range(2):
                                nc.scalar.activation(
                                    out=t_sb[:, h, qo:512], in_=ps_s[:, h, qo:512],
                                    func=AF.Tanh, scale=SCALE / SOFTCAP)
                                nc.scalar.activation(
                                    out=p2[:, h, qo:512], in_=t_sb[:, h, qo:512], func=AF.Exp,
                                    scale=SOFTCAP, bias=ln16_sb)
                                nc.vector.tensor_tensor(
                                    out=sums2[:, h, qo:512], in0=sums2[:, h, qo:512],
                                    in1=p2[:, h, qo:512], op=OP.add)
                        else:
                            nc.scalar.activation(
                                out=_flat(t_sb), in_=_flat(ps_s),
                                func=AF.Tanh, scale=SCALE / SOFTCAP)
                            nc.scalar.activation(
                                out=_flat(p2), in_=_flat(t_sb), func=AF.Exp,
                                scale=SOFTCAP, bias=ln16_sb)
                            if kc == 0:
                                nc.vector.tensor_copy(_flat(sums2), _flat(p2))
                            else:
                                nc.vector.tensor_tensor(
                                    out=_flat(sums2), in0=_flat(sums2), in1=_flat(p2), op=OP.add)
                        p2s[kc] = (p2, qo)

                    def pv(kc, py=py, p2s=p2s, nkc=nkc):
                        p2, qo = p2s[kc]
                        for h in range(2):
                            nc.tensor.matmul(
                                py[h][:, qo:512], v_sb[:, kc, h * VD:(h + 1) * VD], p2[:, h, qo:512],
                                start=(kc == 0), stop=(kc == nkc - 1), skip_group_check=True,
                            )

                    for kc in range(nkc):
                        scores(kc)
                        if kc >= 1:
                            pv(kc - 1)
                        pop_fill()
                    pv(nkc - 1)

                    # epilogue: rowsum + XSA entirely in [d, q] layout
                    rsda = afp.tile([128, 2, 512], F32, tag="fill", name=f"rs_{b}_{qt}")
                    rs2 = rsda[0:1, :, :]
                    for h in range(2):
                        nc.tensor.matmul(rs2[:, h, :], ones_r, sums2[:, h, :],
                                         start=True, stop=True, skip_group_check=True)
                    rr = asm.tile([1, 2, 512], F32, tag="rr")
                    nc.vector.reciprocal_approx_fast(out=_flat(rr), in_=_flat(rs2))
                    dat = afp.tile([128, 2, 512], F32, tag="fill", name=f"da_{b}_{qt}")
                    da = dat[0:1, :, :]
                    for h in range(2):
                        prod = asm.tile([128, 512], F32R, tag="prod", bufs=2)
                        nc.vector.tensor_tensor(
                            out=prod, in0=py[h], in1=vT[b][:, h, qsl], op=OP.mult)
                        nc.tensor.matmul(da[:, h, :], ones_r, prod,
                                         start=True, stop=True, skip_group_check=True)
                    coef = asm.tile([1, 2, 512], F32, tag="coef")
                    for h in range(2):
                        nc.vector.tensor_tensor(
                            out=coef[:, h, :], in0=da[:, h, :], in1=vq[:, h, :], op=OP.mult)
                    for h in range(2):
                        rrb = asm.tile([128, 512], F32, tag="rrb")
                        nc.gpsimd.partition_broadcast(rrb, rr[:, h, :])
                        cb = asm.tile([128, 512], F32, tag="cb")
                        nc.gpsimd.partition_broadcast(cb, coef[:, h, :])
                        tmpc = asm.tile([128, 512], F32, tag="tmpc")
                        nc.vector.tensor_tensor(
                            out=tmpc, in0=cb, in1=vT[b][:, h, qsl], op=OP.mult)
                        nc.vector.tensor_tensor(out=tmpc, in0=py[h], in1=tmpc, op=OP.subtract)
                        nc.vector.tensor_tensor(
                            out=y2T[b][:, h, qsl], in0=tmpc, in1=rrb, op=OP.mult)

                while fillers:
                    pop_fill()

        # standalone Qproj(b0) + vss(b0) + kpe(b0)
        with tc.tile_pool(name="q0ps", bufs=3, space="PSUM") as q0ps:
            for i, u in enumerate([lambda psp: kpe_unit(0, psp)]
                                  + make_qproj_fillers(0) + make_vss_fillers(0)):
                psp = q0ps.tile([128, 2, 512], F32, tag="fill", name=f"q0f{i}")
                u(psp)

        f0 = [lambda psp: kpe_unit(1, psp)] + make_qproj_fillers(1) + make_vss_fillers(1)
        attn(0, f0)
        f1 = make_vload_fillers() + make_wo_fillers(0)
        attn(1, f1)
        with tc.tile_pool(name="w1ps", bufs=2, space="PSUM") as w1ps:
            for i, u in enumerate(make_wo_fillers(1)):
                psp = w1ps.tile([128, 2, 512], F32, tag="fill", name=f"w1f{i}")
                u(psp)


def _build():
    nc = bacc.Bacc("TRN2", target_bir_lowering=False, debug=False, num_devices=NCORES)
    xT = nc.dram_tensor("xT", [DIM, BS], F16, kind="ExternalInput").ap()
    xsT = nc.dram_tensor("xsT", [DIM, STRIP], F16, kind="ExternalInput").ap()
    waqT = nc.dram_tensor("waqT", [DIM, AQR], F16, kind="ExternalInput").ap()
    wakvT = nc.dram_tensor("wakvT", [DIM, KVL], F16, kind="ExternalInput").ap()
    wqbT = nc.dram_tensor("wqbT", [QL, 384], F16, kind="ExternalInput").ap()
    wkbT = nc.dram_tensor("wkbT", [KVL, 256], F16, kind="ExternalInput").ap()
    wvbT = nc.dram_tensor("wvbT", [KVL, 256], F16, kind="ExternalInput").ap()
    woT = nc.dram_tensor("woT", [2 * VD, DIM], F16, kind="ExternalInput").ap()
    cosT = nc.dram_tensor("cosT", [32, S], F16, kind="ExternalInput").ap()
    sinT = nc.dram_tensor("sinT", [32, S], F16, kind="ExternalInput").ap()
    cossT = nc.dram_tensor("cossT", [32, STRIP], F16, kind="ExternalInput").ap()
    sinsT = nc.dram_tensor("sinsT", [32, STRIP], F16, kind="ExternalInput").ap()
    m4h = nc.dram_tensor("m4h", [128, 4 * 2 * 512], F16, kind="ExternalInput").ap()
    out_d = nc.dram_tensor("out", [BS, DIM], F16, kind="ExternalOutput").ap()
    io = (xT, xsT, waqT, wakvT, wqbT, wkbT, wvbT, woT,
          cosT, sinT, cossT, sinsT, m4h, out_d)
    with tile.TileContext(nc) as tc:
        _emit(nc, tc, io)
    nc.compile()
    return nc


def _prep_inputs(inputs):
    f16 = np.float16
    x = np.asarray(inputs["x"], np.float32)
    wq_a = np.asarray(inputs["wq_a_w"], np.float32)
    q_norm = np.asarray(inputs["q_norm_w"], np.float32)
    wq_b = np.asarray(inputs["wq_b_w"], np.float32)
    q_gain = np.asarray(inputs["q_gain"], np.float32)
    wkv_a = np.asarray(inputs["wkv_a_w"], np.float32)
    kv_norm = np.asarray(inputs["kv_norm_w"], np.float32)
    wkv_b = np.asarray(inputs["wkv_b_w"], np.float32)
    wo = np.asarray(inputs["wo_w"], np.float32)
    cos = np.asarray(inputs["freqs_cos"], np.float32)
    sin = np.asarray(inputs["freqs_sin"], np.float32)
    mask = np.asarray(inputs["mask"], np.float32)

    xT = np.ascontiguousarray(x.reshape(BS, DIM).T.astype(f16))

    kpe_rows = wkv_a[KVL:KVL + ROPE]
    kpe_eo = np.concatenate([kpe_rows[0::2], kpe_rows[1::2]], 0)
    waqT = np.ascontiguousarray(np.concatenate([wq_a, kpe_eo], 0).T.astype(f16))
    wakvT = np.ascontiguousarray(wkv_a[:KVL].T.astype(f16))

    qb = wq_b * q_norm[None, :]
    kb = wkv_b * kv_norm[None, :]

    per_core = []
    for c in range(NCORES):
        h0, h1 = 2 * c, 2 * c + 1
        rows = []
        pe_rows = []
        for h in (h0, h1):
            base = h * QKD
            rows.append(qb[base:base + NOPE])
            pe = qb[base + NOPE:base + QKD]
            pe_eo = np.concatenate([pe[0::2], pe[1::2]], 0) * q_gain[h]
            pe_rows.append(pe_eo)
        wqb_c = np.concatenate(rows + pe_rows, 0)  # [384, 768]
        wkb_c = np.concatenate([kb[h * (NOPE + VD): h * (NOPE + VD) + NOPE] for h in (h0, h1)], 0)
        wvb_c = np.concatenate([kb[h * (NOPE + VD) + NOPE: (h + 1) * (NOPE + VD)] for h in (h0, h1)], 0)
        wo_c = wo[:, c * 2 * VD:(c + 1) * 2 * VD]
        pos = (c % 4) * STRIP
        per_core.append(dict(
            wqbT=np.ascontiguousarray(wqb_c.T.astype(f16)),
            wkbT=np.ascontiguousarray(wkb_c.T.astype(f16)),
            wvbT=np.ascontiguousarray(wvb_c.T.astype(f16)),
            woT=np.ascontiguousarray(wo_c.T.astype(f16)),
            xsT=np.ascontiguousarray(xT[:, c * STRIP:(c + 1) * STRIP]),
            cossT=np.ascontiguousarray(cos.T[:, pos:pos + STRIP].astype(f16)),
            sinsT=np.ascontiguousarray(sin.T[:, pos:pos + STRIP].astype(f16)),
        ))

    cosT = np.ascontiguousarray(cos.T.astype(f16))
    sinT = np.ascontiguousarray(sin.T.astype(f16))
    mt = np.maximum(mask[:512, :512].T, -30000.0)  # [k, q], fp16-safe
    m4h = np.stack([mt[d * 128:(d + 1) * 128] for d in range(4)], 1)  # [128,4,512]
    m4h = np.repeat(m4h[:, :, None, :], 2, axis=2)  # [128,4,2,512]
    m4h = np.ascontiguousarray(m4h.reshape(128, 4 * 2 * 512).astype(f16))

    shared = dict(xT=xT, waqT=waqT, wakvT=wakvT, cosT=cosT, sinT=sinT, m4h=m4h)
    return [dict(shared, **pc) for pc in per_core]


_NC_CACHE = {}


def kernel(**inputs):
    if "nc" not in _NC_CACHE:
        _NC_CACHE["nc"] = _build()
    nc = _NC_CACHE["nc"]
    in_maps = _prep_inputs(inputs)
    res = run_bass_kernel_spmd(nc, in_maps, core_ids=list(range(NCORES)))
    out = res.results[0]["out"].astype(np.float64)
    for r in res.results[1:]:
        out += r["out"].astype(np.float64)
    return out.astype(np.float32).reshape(B, S, DIM)
